# revision 9
# baseline (speedup 1.0000x reference)
"""Trainium2 Bass kernel for nn_EnergyACM (GNN message passing), 8 NeuronCores.

All 12 spmm ops in the reference collapse algebraically: every edge value is a
per-row function of degree (a_vals = 1/deg(row); l_vals: diag 1-1/deg,
off-diag -1/deg), so spmm(vals^k, X) = f_k(d)*S(X) + g_k(d)*X where S is the
plain no-self-loop neighbor sum, and S commutes with the dense 128x128 weight
matmuls. The whole network therefore needs only 4 gather primitives in 2
rounds:
  round 1: Q0 = S(h),     Q1 = S(d*h)          gather table T1 = [h | d*h]
  round 2: Q1s = S(d*Sh), Q2 = S(d^2*(Sh+h))   gather table T2
then 8 dense matmuls + relu accumulation reproduce all 4 layers:
  Su1 = Q0-Q1s-Q1, Sv1 = Q1s+Q1, Su2 = Q2+Q0-2*Q1, Sv2 = Q2.

Sharding: nodes range-sharded over 8 cores (12500 each, padded to 12544).
Each core computes only its own h shard; both gather tables are exchanged via
AllGather (T1 = [h | d*h] after phase A2, T2 after round 1). Gathers use
dma_gather (int16 idx -> 4 table windows of 32768 rows) rotated across 4 SWDGE
queues (num_swdge_queues=4) so Q7 descriptor generation overlaps ring drain —
single-queue gathers serialize at ~7.5us/1024 idx, 4-queue at ~5.3us. Segment
sums are one-hot fp8 matmuls accumulating in PSUM, emitted feature-major (Q^T)
so downstream dense matmuls need no transposes.
"""
import numpy as np
import ml_dtypes

import concourse.bass as bass
import concourse.mybir as mybir
import concourse.tile as tile
from concourse import library_config, library_overlay
from concourse.bass_utils import run_bass_kernel_spmd
from concourse.masks import make_identity
from concourse.vector_clock import ScopedClock, VectorClock

# ---------------------------------------------------------------- constants
N_NODES = 100000
IN_CH = 256
HID = 128
OUT = 64
N_LAYERS = 4
N_CORES = 8
NSH = N_NODES // N_CORES          # 12500 real nodes per core
NT = 98                           # 128-row tiles per core
NPAD = NT * 128                   # 12544 padded nodes per core
V = NPAD * N_CORES                # 100352 padded table rows
WIN = 32768                       # int16-addressable window of table rows
N_WIN = -(-V // WIN)              # 4
GROUP = 2                         # row tiles per gather bucket
NG = NT // GROUP                  # 49 groups
GW = GROUP * 128                  # 256-wide group slabs
A2SL = 448                        # phase-A2 slab width (28*448 = NPAD)
F32 = mybir.dt.float32
BF16 = mybir.dt.bfloat16
FP8 = mybir.dt.float8e4
I16 = mybir.dt.int16
RELU = mybir.ActivationFunctionType.Relu

_tilefix_applied = [False]
_cache = {}


# ------------------------------------------------------------- walrus fixes
def _apply_tilefix():
    """This walrus build rejects >1 sem wait per instruction (0 on InstDrain):
    patch the TileContext final drain; extra waits are spilled by
    _split_multiwait after tracing."""
    if _tilefix_applied[0]:
        return
    _tilefix_applied[0] = True

    def _drain_and_barrier(self, tick_clock, wait_clock):
        gc = tick_clock.global_clock
        ticks = list(gc)
        for i, v in enumerate(ticks):
            if v > 0:
                sub = [0] * len(ticks)
                sub[i] = v
                nop = self.nc.sync.nop()
                wait_clock.add_sem_waits(
                    nop.ins, ScopedClock({None: VectorClock(sub)}))
        self.nc.sync.drain()
        self.nc.all_engine_barrier()
        popped = self.nc._tile_sem_poison_stack.pop()
        assert popped is self._sem_poison
        self.nc.clear_and_free_semaphores(list(self.sems.allocated().values()))
        self.nc.all_engine_barrier()

    tile.TileContext._drain_and_barrier = _drain_and_barrier


def _split_multiwait(nc):
    import bass_rust
    uid = [0]
    for fn in nc.m.functions:
        for bb in fn.blocks:
            out = []
            changed = False
            for inst in bb.instructions:
                si = inst.sync_info
                waits = list(si.on_wait) if si and si.on_wait else []
                limit = 0 if type(inst).__name__ == "InstDrain" else 1
                if len(waits) > limit:
                    keep = waits[len(waits) - limit:] if limit else []
                    for w in waits[:len(waits) - limit]:
                        uid[0] += 1
                        nop = mybir.InstNoOp(
                            name=f"WSPLIT-{uid[0]}", ins=[], outs=[])
                        nop.engine = inst.engine
                        nop.sync_info = bass_rust.SyncInfo(
                            on_wait=[w], on_update=[])
                        out.append(nop)
                    inst.sync_info = bass_rust.SyncInfo(
                        on_wait=keep,
                        on_update=list(si.on_update) if si.on_update else [])
                    changed = True
                out.append(inst)
            if changed:
                bb.instructions = out


# ---------------------------------------------------------------- host prep
def _roundup(a, m):
    return -(-a // m) * m


def _preprocess(edge_index):
    row = np.asarray(edge_index[0], dtype=np.int64)
    col = np.asarray(edge_index[1], dtype=np.int64)

    k = row // NSH
    lr = row - k * NSH
    t = lr >> 7
    r = lr & 127
    pg = (col // NSH) * NPAD + (col % NSH)
    c = pg // WIN
    idx16 = (pg - c * WIN).astype(np.int16)

    sec_id = (k * NT + t) * N_WIN + c
    counts = np.bincount(sec_id, minlength=N_CORES * NT * N_WIN)
    counts = counts.reshape(N_CORES, NT, N_WIN)
    caps = _roundup(counts.max(axis=0), 128).astype(np.int64)   # [NT, N_WIN]

    # processing order: (group, window, tile-in-group)
    rank = np.empty((NT, N_WIN), np.int64)
    starts = np.empty((NT, N_WIN), np.int64)
    pos0 = 0
    i = 0
    for g in range(NG):
        for w in range(N_WIN):
            for ti in range(GROUP):
                tt = g * GROUP + ti
                rank[tt, w] = i
                starts[tt, w] = pos0
                pos0 += caps[tt, w]
                i += 1
    p_tot = int(pos0)
    assert p_tot % 128 == 0

    blen = np.zeros((NG, N_WIN), np.int64)
    boff = np.zeros((NG, N_WIN), np.int64)
    for g in range(NG):
        for w in range(N_WIN):
            blen[g, w] = caps[g * GROUP:(g + 1) * GROUP, w].sum()
            boff[g, w] = starts[g * GROUP, w]

    tile_first, tile_last = {}, {}
    for tt in range(NT):
        ws = [w for w in range(N_WIN) if caps[tt, w] > 0]
        assert ws, f"row tile {tt} has no edges"
        tile_first[tt], tile_last[tt] = ws[0], ws[-1]

    # per-core blobs
    edge_rank = rank[t, c]
    sort_idx = np.lexsort((edge_rank, k))
    k_s = k[sort_idx]
    rank_s = edge_rank[sort_idx]
    idx_s = idx16[sort_idx]
    r_s = r[sort_idx]

    cap_rank = caps.reshape(-1)[np.argsort(rank.reshape(-1), kind="stable")]
    start_rank = np.zeros(NT * N_WIN, np.int64)
    start_rank[1:] = np.cumsum(cap_rank)[:-1]

    idxblobs, ohblobs = [], []
    for core in range(N_CORES):
        m = k_s == core
        rk = rank_s[m]
        within = np.arange(rk.shape[0], dtype=np.int64)
        sec_begin = np.zeros(rk.shape[0], np.int64)
        brk = np.flatnonzero(np.diff(rk)) + 1
        sec_begin[brk] = within[brk]
        np.maximum.accumulate(sec_begin, out=sec_begin)
        pos = start_rank[rk] + (within - sec_begin)

        idxpad = np.zeros(p_tot, np.int16)
        idxpad[pos] = idx_s[m]
        rpad = np.full(p_tot, -1, np.int64)
        rpad[pos] = r_s[m]

        idxw = np.zeros((16, p_tot // 16), np.int16)
        for g in range(NG):
            for w in range(N_WIN):
                L = int(blen[g, w])
                if L == 0:
                    continue
                s = int(boff[g, w])
                idxw[:, s // 16:(s + L) // 16] = \
                    idxpad[s:s + L].reshape(L // 16, 16).T
        idxblobs.append(np.ascontiguousarray(np.tile(idxw, (8, 1))))

        oh = np.zeros((128, p_tot), ml_dtypes.float8_e4m3)
        pv = np.flatnonzero(rpad >= 0)
        oh[pv & 127, ((pv >> 7) << 7) + rpad[pv]] = 1.0
        ohblobs.append(oh)

    meta = dict(caps=caps, blen=blen, boff=boff, starts=starts,
                tile_first=tile_first, tile_last=tile_last, p_tot=p_tot)
    return meta, idxblobs, ohblobs


# ---------------------------------------------------------------- device IR
def _build_nc(meta, b1_nonzero):
    _apply_tilefix()
    caps = meta["caps"]
    blen = meta["blen"]
    boff = meta["boff"]
    starts = meta["starts"]
    tile_first = meta["tile_first"]
    tile_last = meta["tile_last"]
    p_tot = meta["p_tot"]
    wins = [min(WIN, V - w * WIN) for w in range(N_WIN)]

    nc = bass.Bass(target_bir_lowering=False, num_swdge_queues=4)
    xTo = nc.dram_tensor("xTo", [IN_CH, NPAD], BF16, kind="ExternalInput")
    wlin1 = nc.dram_tensor("wlin1", [IN_CH, HID], BF16, kind="ExternalInput")
    w1 = nc.dram_tensor("w1", [N_LAYERS, HID, HID], F32, kind="ExternalInput")
    w2 = nc.dram_tensor("w2", [N_LAYERS, HID, HID], F32, kind="ExternalInput")
    wlin2 = nc.dram_tensor("wlin2", [HID, OUT], F32, kind="ExternalInput")
    b2 = nc.dram_tensor("b2", [OUT, 1], F32, kind="ExternalInput")
    b1col = nc.dram_tensor("b1col", [HID, 1], F32, kind="ExternalInput")
    b1row = nc.dram_tensor("b1row", [1, HID], F32, kind="ExternalInput")
    drow = nc.dram_tensor("drow", [1, NPAD], F32, kind="ExternalInput")
    mrow = nc.dram_tensor("mrow", [1, NPAD], F32, kind="ExternalInput")
    idxblob = nc.dram_tensor("idxblob", [128, p_tot // 16], I16,
                             kind="ExternalInput")
    ohblob = nc.dram_tensor("ohblob", [128, p_tot], FP8, kind="ExternalInput")
    outT = nc.dram_tensor("outT", [OUT, NPAD], F32, kind="ExternalOutput")
    dbg = nc.dram_tensor("dbg", [128, NPAD], F32, kind="ExternalOutput")
    import os
    phase_mode = os.environ.get("KERNEL_PHASES", "full")
    no_mm = os.environ.get("KERNEL_NOMM", "") == "1"
    no_gather = os.environ.get("KERNEL_NOGATHER", "") == "1"

    with tile.TileContext(nc) as tc:
        with (
            tc.tile_pool(name="const", bufs=1) as cpool,
            tc.tile_pool(name="resid", bufs=1) as rpool,
            tc.tile_pool(name="sbuf", bufs=3) as spool,
            tc.tile_pool(name="gath", bufs=2) as gpool,
            tc.tile_pool(name="fsc", bufs=1) as fsc,
            tc.tile_pool(name="fzi", bufs=2) as fzi,
            tc.tile_pool(name="psum", bufs=4, space="PSUM") as ppool,
            tc.tile_pool(name="psacc", bufs=1, space="PSUM") as papool,
            tc.tile_pool(name="dram", bufs=1, space="DRAM") as dpool,
        ):
            nc.gpsimd.load_library(library_config.mlp)

            # constants
            wl_a = cpool.tile([128, HID], BF16, tag="wl_a")
            nc.sync.dma_start(out=wl_a[:], in_=wlin1[0:128, :])
            wl_b = cpool.tile([128, HID], BF16, tag="wl_b")
            nc.sync.dma_start(out=wl_b[:], in_=wlin1[128:256, :])
            w1t = cpool.tile([128, N_LAYERS * HID], F32, tag="w1t")
            w2t = cpool.tile([128, N_LAYERS * HID], F32, tag="w2t")
            for l in range(N_LAYERS):
                nc.sync.dma_start(out=w1t[:, l * HID:(l + 1) * HID],
                                  in_=w1[l, :, :])
                nc.sync.dma_start(out=w2t[:, l * HID:(l + 1) * HID],
                                  in_=w2[l, :, :])
            wl2 = cpool.tile([128, OUT], F32, tag="wl2")
            nc.sync.dma_start(out=wl2[:], in_=wlin2[:])
            b2t = cpool.tile([OUT, 1], F32, tag="b2t")
            nc.sync.dma_start(out=b2t[:], in_=b2[:])
            b1c = cpool.tile([HID, 1], F32, tag="b1c")
            nc.sync.dma_start(out=b1c[:], in_=b1col[:])
            ones = cpool.tile([1, 128], F32, tag="ones")
            nc.gpsimd.memset(ones[:], 1.0)
            ident = cpool.tile([128, 128], BF16, tag="ident")
            make_identity(nc, ident[:])
            if b1_nonzero:
                b1r = cpool.tile([1, HID], F32, tag="b1r")
                nc.sync.dma_start(out=b1r[:], in_=b1row[:])
                pb = ppool.tile([128, HID], F32, space="PSUM", tag="pp")
                nc.tensor.matmul(out=pb[:], lhsT=ones[:], rhs=b1r[:],
                                 start=True, stop=True)
                b1bc = cpool.tile([128, HID], F32, tag="b1bc")
                nc.vector.tensor_copy(b1bc[:], pb[:])

            # DRAM scratch
            T1own = dpool.tile([NPAD, 2 * HID], BF16, tag="T1own")
            T1full = dpool.tile([V, 2 * HID], BF16, tag="T1full",
                                addr_space="Shared")
            T2own = dpool.tile([NPAD, 2 * HID], BF16, tag="T2own")
            T2full = dpool.tile([V, 2 * HID], BF16, tag="T2full",
                                addr_space="Shared")

            # residents
            hT = rpool.tile([128, NPAD], BF16, tag="hT")
            q0T = rpool.tile([128, NPAD], BF16, tag="q0T")
            q1T = rpool.tile([128, NPAD], BF16, tag="q1T")

            # ============ phase A2: own h^T resident (feature-major)
            for s in range(NPAD // A2SL):
                cs = s * A2SL
                xoa = spool.tile([128, A2SL], BF16, tag="xoa")
                nc.sync.dma_start(out=xoa[:], in_=xTo[0:128, cs:cs + A2SL])
                xob = spool.tile([128, A2SL], BF16, tag="xob")
                nc.sync.dma_start(out=xob[:], in_=xTo[128:256, cs:cs + A2SL])
                ph2 = ppool.tile([128, A2SL], F32, space="PSUM", tag="pp")
                nc.tensor.matmul(out=ph2[:], lhsT=wl_a[:], rhs=xoa[:],
                                 start=True, stop=False)
                nc.tensor.matmul(out=ph2[:], lhsT=wl_b[:], rhs=xob[:],
                                 start=False, stop=True)
                nc.scalar.activation(hT[:, cs:cs + A2SL], ph2[:], RELU,
                                     bias=b1c[:])

            # ============ phase A: own T1 shard [h | d*h] node-major + AG
            for t in range(NT):
                c0 = t * 128
                dr_t = spool.tile([1, 128], F32, tag="dr_t")
                nc.sync.dma_start(out=dr_t[:], in_=drow[:, c0:c0 + 128])
                pD = ppool.tile([128, 128], F32, space="PSUM", tag="pp")
                nc.tensor.matmul(out=pD[:], lhsT=ones[:], rhs=dr_t[:],
                                 start=True, stop=True)
                h_sl = hT[:, c0:c0 + 128]
                dh = spool.tile([128, 128], BF16, tag="dsh")
                nc.vector.tensor_mul(dh[:], pD[:], h_sl)
                ptr1 = ppool.tile([128, 128], BF16, space="PSUM", tag="pp")
                nc.tensor.transpose(out=ptr1[:], in_=h_sl,
                                    identity=ident[:])
                ptr2 = ppool.tile([128, 128], BF16, space="PSUM", tag="pp")
                nc.tensor.transpose(out=ptr2[:], in_=dh[:],
                                    identity=ident[:])
                t1t = spool.tile([128, 2 * HID], BF16, tag="t1t")
                nc.vector.tensor_copy(t1t[:, 0:HID], ptr1[:])
                nc.vector.tensor_copy(t1t[:, HID:2 * HID], ptr2[:])
                nc.sync.dma_start(out=T1own[c0:c0 + 128, :], in_=t1t[:])

            nc.gpsimd.collective_compute(
                "AllGather", mybir.AluOpType.bypass,
                replica_groups=[list(range(N_CORES))],
                ins=[T1own.opt()], outs=[T1full.opt()])

            # ============ shared gather + segment-sum round body
            _reg_cache = {}
            _qrot = [0]

            def _len_reg(L):
                if L not in _reg_cache:
                    _reg_cache[L] = nc.gpsimd.to_reg(L)
                return _reg_cache[L]

            def _next_q():
                q = _qrot[0]
                _qrot[0] = (q + 1) % 4
                return q

            def gather_round(table, dst_a, dst_b, fused_f):
                for g in range(NG):
                    psa = [papool.tile([128, 128], F32, space="PSUM",
                                       tag=f"psqa{i}", name=f"psa{i}")
                           for i in range(GROUP)]
                    psb = [papool.tile([128, 128], F32, space="PSUM",
                                       tag=f"psqb{i}", name=f"psb{i}")
                           for i in range(GROUP)]
                    if no_mm:
                        for i in range(GROUP):
                            nc.tensor.matmul(out=psa[i][:], lhsT=ident[:],
                                             rhs=ident[:], start=True,
                                             stop=True, skip_group_check=True)
                            nc.tensor.matmul(out=psb[i][:], lhsT=ident[:],
                                             rhs=ident[:], start=True,
                                             stop=True, skip_group_check=True)
                    for w in range(N_WIN):
                        L = int(blen[g, w])
                        if L == 0:
                            continue
                        off = int(boff[g, w])
                        nch = L // 128
                        idx_t = gpool.tile([128, L // 16], I16, tag="idx")
                        nc.sync.dma_start(
                            out=idx_t[:],
                            in_=idxblob[:, off // 16:(off + L) // 16])
                        g_t = gpool.tile([128, nch, 2 * HID], BF16, tag="g")
                        if no_gather:
                            nc.gpsimd.memset(g_t[:], 0.25)
                        else:
                            # SWDGE desc ring holds ~128 descs/engine:
                            # split gathers to <=1024 idx per call
                            for a in range(0, L, 1024):
                                Ls = min(1024, L - a)
                                nc.gpsimd.dma_gather(
                                    out_ap=g_t[:, a // 128:(a + Ls) // 128, :],
                                    in_ap=table[w * WIN:w * WIN + wins[w], :],
                                    idxs_ap=idx_t[:, a // 16:(a + Ls) // 16],
                                    num_idxs=Ls, num_idxs_reg=_len_reg(Ls),
                                    elem_size=2 * HID, queue_num=_next_q())
                        oh_t = gpool.tile([128, L], FP8, tag="oh")
                        nc.sync.dma_start(
                            out=oh_t[:], in_=ohblob[:, off:off + L])
                        for ti in range(GROUP):
                            if no_mm:
                                break
                            tt = g * GROUP + ti
                            cap = int(caps[tt, w])
                            if cap == 0:
                                continue
                            sec = int(starts[tt, w]) - off
                            for j in range(cap // 128):
                                ch = (sec // 128) + j
                                st = (w == tile_first[tt]) and j == 0
                                sp = (w == tile_last[tt]) and \
                                    j == cap // 128 - 1
                                sl = slice(sec + j * 128, sec + (j + 1) * 128)
                                nc.tensor.matmul(
                                    out=psa[ti][:],
                                    lhsT=g_t[:, ch, 0:HID],
                                    rhs=oh_t[:, sl],
                                    start=st, stop=sp, skip_group_check=True)
                                nc.tensor.matmul(
                                    out=psb[ti][:],
                                    lhsT=g_t[:, ch, HID:2 * HID],
                                    rhs=oh_t[:, sl],
                                    start=st, stop=sp, skip_group_check=True)
                    if fused_f is None:
                        c0 = g * GW
                        for i in range(GROUP):
                            nc.vector.tensor_copy(
                                dst_a[:, c0 + i * 128:c0 + (i + 1) * 128],
                                psa[i][:])
                            nc.vector.tensor_copy(
                                dst_b[:, c0 + i * 128:c0 + (i + 1) * 128],
                                psb[i][:])
                    else:
                        qa = fzi.tile([128, GW], F32, tag="qa")
                        qb = fzi.tile([128, GW], F32, tag="qb")
                        for i in range(GROUP):
                            nc.vector.tensor_copy(
                                qa[:, i * 128:(i + 1) * 128], psa[i][:])
                            nc.vector.tensor_copy(
                                qb[:, i * 128:(i + 1) * 128], psb[i][:])
                        fused_f(g, qa, qb)

            # ============ phase C: round-1 gathers -> q0T, q1T (bf16)
            if phase_mode in ("c", "d", "full"):
                gather_round(T1full, q0T, q1T, None)

            if phase_mode == "a":
                nc.gpsimd.dma_start(out=dbg[:], in_=hT[:])
            else:
                nc.gpsimd.dma_start(out=dbg[:], in_=q0T[:])
            if phase_mode in ("a", "c"):
                zot = spool.tile([OUT, A2SL], F32, tag="zot")
                nc.gpsimd.memset(zot[:], 0.0)
                for s_i in range(NPAD // A2SL):
                    nc.sync.dma_start(
                        out=outT[:, s_i * A2SL:(s_i + 1) * A2SL], in_=zot[:])

            # ============ phase D: build T2own, AllGather -> T2full
            for t in (range(NT) if phase_mode in ("d", "full") else []):
                c0 = t * 128
                dr_t = spool.tile([1, 128], F32, tag="dr_t")
                nc.sync.dma_start(out=dr_t[:], in_=drow[:, c0:c0 + 128])
                pD = ppool.tile([128, 128], F32, space="PSUM", tag="pp")
                nc.tensor.matmul(out=pD[:], lhsT=ones[:], rhs=dr_t[:],
                                 start=True, stop=True)
                a_sl = q0T[:, c0:c0 + 128]
                h_sl = hT[:, c0:c0 + 128]
                dsh = spool.tile([128, 128], F32, tag="dsh")
                nc.vector.tensor_mul(dsh[:], pD[:], a_sl)
                ssum = spool.tile([128, 128], F32, tag="ssum")
                nc.vector.tensor_add(ssum[:], a_sl, h_sl)
                nc.vector.tensor_mul(ssum[:], ssum[:], pD[:])
                nc.vector.tensor_mul(ssum[:], ssum[:], pD[:])
                c1b = spool.tile([128, 128], BF16, tag="c1b")
                nc.vector.tensor_copy(c1b[:], dsh[:])
                c2b = spool.tile([128, 128], BF16, tag="c2b")
                nc.vector.tensor_copy(c2b[:], ssum[:])
                ptr1 = ppool.tile([128, 128], BF16, space="PSUM", tag="pp")
                nc.tensor.transpose(out=ptr1[:], in_=c1b[:],
                                    identity=ident[:])
                ptr2 = ppool.tile([128, 128], BF16, space="PSUM", tag="pp")
                nc.tensor.transpose(out=ptr2[:], in_=c2b[:],
                                    identity=ident[:])
                t2t = spool.tile([128, 2 * HID], BF16, tag="t2t")
                nc.vector.tensor_copy(t2t[:, 0:HID], ptr1[:])
                nc.vector.tensor_copy(t2t[:, HID:2 * HID], ptr2[:])
                nc.sync.dma_start(out=T2own[c0:c0 + 128, :], in_=t2t[:])

            if phase_mode in ("d", "full"):
                nc.gpsimd.collective_compute(
                    "AllGather", mybir.AluOpType.bypass,
                    replica_groups=[list(range(N_CORES))],
                    ins=[T2own.opt()], outs=[T2full.opt()])
            if phase_mode == "d":
                t2dbg = spool.tile([128, 2 * HID], BF16, tag="t2dbg")
                nc.sync.dma_start(out=t2dbg[:], in_=T2full[0:128, :])
                zot = spool.tile([OUT, A2SL], F32, tag="zot")
                nc.gpsimd.memset(zot[:], 0.0)
                nc.vector.tensor_copy(zot[:, 0:2 * HID], t2dbg[0:OUT, :])
                for s_i in range(NPAD // A2SL):
                    nc.sync.dma_start(
                        out=outT[:, s_i * A2SL:(s_i + 1) * A2SL], in_=zot[:])

            # ============ phase E + F: round-2 gathers fused with layer math
            def fphase(g, qa, qb):
                cols = slice(g * GW, (g + 1) * GW)

                def ft(tag):
                    return fsc.tile([128, GW], F32, tag=tag, name=f"f_{tag}")

                def fz(tag):
                    return fzi.tile([128, GW], F32, tag=tag, name=f"f_{tag}")

                dr_s = spool.tile([1, GW], F32, tag="dr_s")
                nc.sync.dma_start(out=dr_s[:], in_=drow[:, cols])
                mr_s = spool.tile([1, GW], F32, tag="mr_s")
                nc.sync.dma_start(out=mr_s[:], in_=mrow[:, cols])
                pDf = ppool.tile([128, GW], F32, space="PSUM", tag="pp")
                nc.tensor.matmul(out=pDf[:], lhsT=ones[:], rhs=dr_s[:],
                                 start=True, stop=True)
                D = ft("D")
                nc.vector.tensor_copy(D[:], pDf[:])
                pMf = ppool.tile([128, GW], F32, space="PSUM", tag="pp")
                nc.tensor.matmul(out=pMf[:], lhsT=ones[:], rhs=mr_s[:],
                                 start=True, stop=True)
                M = ft("M")
                nc.vector.tensor_copy(M[:], pMf[:])

                A = q0T[:, cols]
                H = hT[:, cols]
                Q1 = q1T[:, cols]

                D2 = ft("D2"); nc.vector.tensor_mul(D2[:], D[:], D[:])
                D3 = ft("D3"); nc.vector.tensor_mul(D3[:], D2[:], D[:])
                M2 = ft("M2"); nc.vector.tensor_mul(M2[:], M[:], M[:])
                M3 = ft("M3"); nc.vector.tensor_mul(M3[:], M2[:], M[:])

                Ssum = ft("Ssum"); nc.vector.tensor_add(Ssum[:], A, H)
                v1 = fz("v1"); nc.vector.tensor_mul(v1[:], D[:], Ssum[:])
                v2 = ft("v2"); nc.vector.tensor_mul(v2[:], D[:], v1[:])
                v3 = fz("v3"); nc.vector.tensor_mul(v3[:], D[:], v2[:])

                e1 = ft("e1"); e2 = ft("e2")
                nc.vector.tensor_mul(e1[:], D[:], A)
                nc.vector.tensor_mul(e2[:], M[:], H)
                u1 = fz("u1"); nc.vector.tensor_sub(u1[:], e2[:], e1[:])
                nc.vector.tensor_mul(e1[:], D2[:], A)
                nc.vector.tensor_mul(e2[:], M2[:], H)
                u2 = ft("u2"); nc.vector.tensor_add(u2[:], e1[:], e2[:])
                nc.vector.tensor_mul(e1[:], D3[:], A)
                nc.vector.tensor_mul(e2[:], M3[:], H)
                u3 = fz("u3"); nc.vector.tensor_sub(u3[:], e2[:], e1[:])

                Su1 = ft("Su1")
                nc.vector.tensor_sub(Su1[:], A, qa[:])
                nc.vector.tensor_sub(Su1[:], Su1[:], Q1)
                Sv1 = ft("Sv1"); nc.vector.tensor_add(Sv1[:], qa[:], Q1)
                Su2 = ft("Su2")
                nc.vector.tensor_add(Su2[:], qb[:], A)
                nc.vector.tensor_sub(Su2[:], Su2[:], Q1)
                nc.vector.tensor_sub(Su2[:], Su2[:], Q1)

                z2 = fz("z2")
                nc.vector.tensor_add(z2[:], Su1[:], u1[:])
                nc.vector.tensor_mul(z2[:], z2[:], D[:])
                z3 = fz("z3")
                nc.vector.tensor_mul(z3[:], D[:], Sv1[:])
                nc.vector.tensor_mul(e2[:], M[:], v1[:])
                nc.vector.tensor_sub(z3[:], e2[:], z3[:])
                z4 = fz("z4")
                nc.vector.tensor_add(z4[:], Su2[:], u2[:])
                nc.vector.tensor_mul(z4[:], z4[:], D[:])
                z5 = fz("z5")
                nc.vector.tensor_mul(z5[:], D[:], qb[:])
                nc.vector.tensor_mul(e2[:], M[:], v2[:])
                nc.vector.tensor_sub(z5[:], e2[:], z5[:])

                zin = [v1, u1, z2, z3, z4, z5, u3, v3]
                wsel = [(w1t, 0), (w2t, 0), (w1t, 1), (w2t, 1),
                        (w1t, 2), (w2t, 2), (w1t, 3), (w2t, 3)]
                acc = fzi.tile([128, GW], F32, tag="acc")
                for zi in range(8):
                    wt, l = wsel[zi]
                    pz = ppool.tile([128, GW], F32, space="PSUM", tag="pp")
                    nc.tensor.matmul(
                        out=pz[:], lhsT=wt[:, l * HID:(l + 1) * HID],
                        rhs=zin[zi][:], start=True, stop=True)
                    if zi == 0:
                        nc.scalar.activation(acc[:], pz[:], RELU)
                    else:
                        rl = spool.tile([128, GW], F32, tag="rl")
                        nc.scalar.activation(rl[:], pz[:], RELU)
                        nc.vector.tensor_add(acc[:], acc[:], rl[:])
                po = ppool.tile([OUT, GW], F32, space="PSUM", tag="pp")
                nc.tensor.matmul(out=po[:], lhsT=wl2[:], rhs=acc[:],
                                 start=True, stop=True)
                ot = spool.tile([OUT, GW], F32, tag="ot")
                nc.vector.tensor_scalar_add(ot[:], po[:], b2t[:])
                nc.sync.dma_start(out=outT[:, cols], in_=ot[:])

            if phase_mode == "full":
                gather_round(T2full, None, None, fphase)

    library_overlay.lower_extended_insts(nc)
    _split_multiwait(nc)
    return nc


# ---------------------------------------------------------------- kernel()
def kernel(x, W_lin1, b_lin1, W1, W2, W_lin2, b_lin2, edge_index):
    x = np.asarray(x, np.float32)
    W_lin1 = np.asarray(W_lin1, np.float32)
    b_lin1 = np.asarray(b_lin1, np.float32)
    W1 = np.ascontiguousarray(np.asarray(W1, np.float32))
    W2 = np.ascontiguousarray(np.asarray(W2, np.float32))
    W_lin2 = np.asarray(W_lin2, np.float32)
    b_lin2 = np.asarray(b_lin2, np.float32)
    ei = np.asarray(edge_index)

    meta, idxblobs, ohblobs = _preprocess(ei)
    b1nz = bool(np.any(b_lin1 != 0.0))

    import os
    key = (int(np.sum(meta["caps"])), meta["p_tot"], b1nz,
           os.environ.get("KERNEL_PHASES", "full"),
           os.environ.get("KERNEL_NOMM", ""),
           os.environ.get("KERNEL_NOGATHER", ""))
    if key not in _cache:
        _cache[key] = _build_nc(meta, b1nz)
    nc = _cache[key]

    row = np.asarray(ei[0], dtype=np.int64)
    deg = np.bincount(row, minlength=N_NODES).astype(np.float32) + 1.0
    d = 1.0 / deg
    d_pad = np.ones(V, np.float32)
    for k in range(N_CORES):
        d_pad[k * NPAD:k * NPAD + NSH] = d[k * NSH:(k + 1) * NSH]

    xT_pad = np.zeros((IN_CH, V), ml_dtypes.bfloat16)
    xt = np.ascontiguousarray(x.T)
    for k in range(N_CORES):
        xT_pad[:, k * NPAD:k * NPAD + NSH] = \
            xt[:, k * NSH:(k + 1) * NSH].astype(ml_dtypes.bfloat16)

    common = {
        "wlin1": W_lin1.astype(ml_dtypes.bfloat16),
        "w1": W1,
        "w2": W2,
        "wlin2": np.ascontiguousarray(W_lin2 / (2.0 * N_LAYERS)),
        "b2": np.ascontiguousarray(b_lin2[:, None]),
        "b1col": np.ascontiguousarray(b_lin1[:, None]),
        "b1row": np.ascontiguousarray(b_lin1[None, :]),
    }
    in_maps = []
    for k in range(N_CORES):
        dr = d_pad[k * NPAD:(k + 1) * NPAD][None, :].astype(np.float32)
        in_maps.append(dict(
            common,
            xTo=np.ascontiguousarray(xT_pad[:, k * NPAD:(k + 1) * NPAD]),
            drow=np.ascontiguousarray(dr),
            mrow=np.ascontiguousarray(1.0 - dr),
            idxblob=idxblobs[k],
            ohblob=ohblobs[k],
        ))

    res = run_bass_kernel_spmd(nc, in_maps, core_ids=list(range(N_CORES)))
    kernel._last_results = res

    out = np.empty((N_NODES, OUT), np.float32)
    for k in range(N_CORES):
        out[k * NSH:(k + 1) * NSH, :] = res.results[k]["outT"][:, :NSH].T
    return out

